# revision 30
# baseline (speedup 1.0000x reference)
"""DeepSeek MLA prefill (absorbed) on 8 Trainium2 NeuronCores.

v3c: host-folded weights, head-sequential pipelined attention, batched
multi-queue DMA.

- W_uq folded through W_cqkv on host (W_eff = Wq @ W_uq): q for the local
  2 heads is one local GEMM against replicated x — no AllReduce; the first
  collective (o2 AllGather, head 0) fires mid-kernel, hiding the CC entry
  barrier entirely.
- W_o1 folded into V on host (V2[h] = V @ W_o1[h], [4096,128]): the value
  matmul contracts straight to o2 (4x less PE work, no o2 bmm).
- Attention head-sequential: head 0 scores->value->o2->AllGather, then
  head 1 (kv chunks stay SBUF-resident for the second pass); O-projection
  accumulates head-0 k-tiles while head 1's AllGather is in flight.
- Z is a running DVE accumulate over score chunks; one small f32 matmul
  folds it across partitions per head.
- DMA dispatch is serialized per issuing engine (~0.6us each), so
  transfers are batched into few big host-interleaved blocks and spread
  over the three dispatch paths: kv stream on SP, cnt stream on ACT,
  weight prefetches on GpSimd SWDGE.
- Top-k selection is folded in as a count matrix: softmax over gathered
  scores == count-weighted dense softmax against the full kv cache.
"""

import os
import sys

sys.path.insert(0, "/opt/trn_rl_repo")

import numpy as np

import concourse.bass as bass
import concourse.tile as tile
from concourse import bacc, mybir
from concourse.bass_utils import run_bass_kernel_spmd

F32 = mybir.dt.float32
F16 = mybir.dt.float16
NP16 = np.float16

N_CORES = 8
M = 512
HID = 7168
KB4 = HID // 512         # 14 blocks of 4 k-tiles
QL = 384                 # 2 local heads x (128 nope + 64 pe)
H_LOC = 2
S_KV = 4096
NSC = S_KV // 128        # 32 key chunks
OUT_C = HID // N_CORES   # 896
SM_SCALE = 1.0 / float(np.sqrt(np.float32(576)))
N_WARM = 20


def build_program():
    nc = bacc.Bacc("TRN2", target_bir_lowering=False, debug=False,
                   num_devices=N_CORES)

    xt = nc.dram_tensor("xt", [KB4, 128, 2048], F16, kind="ExternalInput")
    wef = nc.dram_tensor("wef", [KB4, 128, 4 * QL], F16,
                         kind="ExternalInput")
    wqk = nc.dram_tensor("wqk", [H_LOC, 128, 512], F16, kind="ExternalInput")
    # per chunk [128, 640]: 4 d-tiles + pe tile (rows 0:64 pe, 64:128 dup)
    kvt = nc.dram_tensor("kvt", [NSC, 128, 640], F16, kind="ExternalInput")
    cnt = nc.dram_tensor("cnt", [NSC // 2, 128, 1024], F16,
                         kind="ExternalInput")
    v2 = nc.dram_tensor("v2", [H_LOC, 128, S_KV], F16, kind="ExternalInput")
    wop = nc.dram_tensor("wop", [128, 16 * OUT_C], F16, kind="ExternalInput")
    outT = nc.dram_tensor("outT", [7, 128, M], F16, kind="ExternalOutput")

    rg = [list(range(N_CORES))]

    with tile.TileContext(nc) as tc, \
            nc.allow_low_precision(reason="fp16 matmul pipeline"):
        with tc.tile_pool(name="dram", bufs=1, space="DRAM") as dram:
            o2_loc = [dram.tile([128, M], F16, name=f"o2loc{h}")
                      for h in range(H_LOC)]
            o2_all = [dram.tile([128 * N_CORES, M], F16, name=f"o2all{h}",
                                addr_space="Shared") for h in range(H_LOC)]

            per_cm = tc.tile_pool(name="per", bufs=1)
            per = per_cm.__enter__()
            wqkt = []
            for h in range(H_LOC):
                wh = per.tile([128, 512], F16, name=f"wqk{h}")
                nc.sync.dma_start(wh[:], wqk[h])
                wqkt.append(wh)
            kvda = per.tile([128, NSC * 640], F16, name="kvda")
            pt = per.tile([128, NSC * M], F16, name="pt")
            qa = [[None] * 5 for _ in range(H_LOC)]
            zacc = [per.tile([128, M], F32, name=f"zacc{h}")
                    for h in range(H_LOC)]
            ones_col_f = per.tile([128, 1], F32, name="ones_col_f")
            nc.vector.memset(ones_col_f[:], 1.0)
            ones_row_f = per.tile([1, 128], F32, name="ones_row_f")
            nc.vector.memset(ones_row_f[:], 1.0)
            ones_row = per.tile([1, 128], F16, name="ones_row")
            nc.vector.tensor_copy(ones_row[:], ones_row_f[:])
            z_sb = [per.tile([1, M], F32, name=f"z{h}")
                    for h in range(H_LOC)]
            rz = [per.tile([1, M], F16, name=f"rz{h}") for h in range(H_LOC)]
            zb_sb = [per.tile([128, M], F16, name=f"zs{h}")
                     for h in range(H_LOC)]
            v2t = [per.tile([128, S_KV], F16, name=f"v2t{h}")
                   for h in range(H_LOC)]
            wopa = per.tile([128, 16 * OUT_C], F16, name="wopa")
            o2s = [per.tile([128, M], F16, name=f"o2s{h}")
                   for h in range(H_LOC)]
            o2at = [per.tile([128, 8 * M], F16, name=f"o2at{h}")
                    for h in range(H_LOC)]

            # ---------------- fused q GEMM (stage 1+2) --------------------
            qch = []
            with (
                tc.tile_pool(name="s12w", bufs=1) as s12w,
                tc.tile_pool(name="s12x", bufs=5) as s12x,
                tc.tile_pool(name="s12e", bufs=5) as s12e,
                tc.tile_pool(name="ps12", bufs=1, space="PSUM") as ps12,
            ):
                warm = s12w.tile([128, 64], F32, name="warm")
                nc.vector.memset(warm[:], 0.0)
                wps = ps12.tile([1, 64], F32, name="wps", tag="wps")
                for i in range(N_WARM):
                    nc.tensor.matmul(wps[:], warm[:, 0:1], warm[:],
                                     start=(i == 0), stop=(i == N_WARM - 1),
                                     skip_group_check=True)
                acc12 = [ps12.tile([128, M], F32, name=f"a12_{p}",
                                   tag=f"a12_{p}") for p in range(3)]
                for k4 in range(KB4):
                    xk = s12x.tile([128, 2048], F16, name="xk", tag="xk")
                    nc.sync.dma_start(xk[:], xt[k4])
                    ek = s12e.tile([128, 4 * QL], F16, name="ek", tag="ek")
                    nc.sync.dma_start(ek[:], wef[k4])
                    for q in range(4):
                        for p in range(3):
                            nc.tensor.matmul(
                                acc12[p][:],
                                ek[:, q * QL + p * 128:q * QL + (p + 1) * 128],
                                xk[:, q * 512:(q + 1) * 512],
                                start=(k4 == 0 and q == 0),
                                stop=(k4 == KB4 - 1 and q == 3))
                for p in range(3):
                    qc = per.tile([128, M], F16, name=f"qch{p}")
                    nc.vector.tensor_copy(qc[:], acc12[p][:])
                    qch.append(qc)
            for h in range(H_LOC):
                qa[h][4] = qch[2][h * 64:(h + 1) * 64, :]

            # ---------------- q absorb (stage 3) --------------------------
            with tc.tile_pool(name="ps3", bufs=2, space="PSUM") as ps3:
                for h in range(H_LOC):
                    for c in range(4):
                        acc = ps3.tile([128, M], F32, name="acc3", tag="acc3")
                        nc.tensor.matmul(
                            acc[:], wqkt[h][:, c * 128:(c + 1) * 128],
                            qch[h][:], start=True, stop=True)
                        qb = per.tile([128, M], F16, name=f"qa{h}_{c}")
                        nc.vector.tensor_copy(qb[:], acc[:])
                        qa[h][c] = qb

            # ---------------- attention + O path, head-sequential ---------
            with (
                tc.tile_pool(name="cnts", bufs=3) as cnts,
                tc.tile_pool(name="exps", bufs=5) as exps,
                tc.tile_pool(name="psS", bufs=4, space="PSUM") as psS,
                tc.tile_pool(name="psV", bufs=1, space="PSUM") as psV,
                tc.tile_pool(name="psZ", bufs=1, space="PSUM") as psZ,
                tc.tile_pool(name="psB", bufs=1, space="PSUM") as psB,
            ):
                for h in range(H_LOC):
                    cc2 = None
                    for sc in range(NSC):
                        if h == 0:
                            # stream kv chunk into the resident tile via SP;
                            # head 1 reuses it from SBUF
                            nc.sync.dma_start(
                                kvda[:, sc * 640:(sc + 1) * 640], kvt[sc])
                        if sc % 2 == 0:
                            # cnt pairs stream on the ACT hwdge queue
                            cc2 = cnts.tile([128, 1024], F16, name="cc",
                                            tag="cc")
                            nc.scalar.dma_start(cc2[:], cnt[sc // 2])
                        cc = cc2[:, (sc % 2) * 512:(sc % 2 + 1) * 512]
                        acc = psS.tile([128, M], F32, name="accS", tag="accS")
                        for j in range(5):
                            if j < 4:
                                lhsT = kvda[:, sc * 640 + j * 128:
                                            sc * 640 + (j + 1) * 128]
                                rhs = qa[h][j][:]
                            else:
                                lhsT = kvda[h * 64:(h + 1) * 64,
                                            sc * 640 + 512:sc * 640 + 640]
                                rhs = qa[h][4]
                            nc.tensor.matmul(
                                acc[:], lhsT, rhs,
                                start=(j == 0), stop=(j == 4))
                        ex = exps.tile([128, M], F16, name="ex", tag="ex")
                        nc.scalar.activation(
                            ex[:], acc[:], mybir.ActivationFunctionType.Exp,
                            scale=SM_SCALE)
                        psl = pt[:, sc * M:(sc + 1) * M]
                        nc.vector.tensor_mul(psl, ex[:], cc)
                        if sc == 0:
                            nc.vector.tensor_copy(zacc[h][:], psl)
                        else:
                            nc.vector.tensor_add(zacc[h][:], zacc[h][:], psl)
                        if h == 0 and sc >= 16 and sc % 2 == 0:
                            # one v2 prefetch slice between cnt pairs on
                            # the ACT queue (8 slices over sc 16..30)
                            s8 = (sc - 16) // 2
                            hh, s4 = s8 // 4, s8 % 4
                            nc.scalar.dma_start(
                                v2t[hh][:, s4 * 1024:(s4 + 1) * 1024],
                                v2[hh][:, s4 * 1024:(s4 + 1) * 1024])

                    # value phase; the Z fold rides along once the DVE
                    # accumulate chain has drained
                    o_ps = psV.tile([128, M], F32, name=f"op{h}",
                                    tag=f"vh{h}")
                    zp = psZ.tile([1, M], F32, name="zp", tag="zp")
                    for sc in range(NSC):
                        nc.tensor.matmul(
                            o_ps[:], v2t[h][:, sc * 128:(sc + 1) * 128],
                            pt[:, sc * M:(sc + 1) * M],
                            start=(sc == 0), stop=(sc == NSC - 1),
                            skip_group_check=True)
                        if sc == 4:
                            nc.tensor.matmul(zp[:], ones_col_f[:],
                                             zacc[h][:], start=True,
                                             stop=True,
                                             skip_group_check=True)
                            nc.vector.tensor_copy(z_sb[h][:], zp[:])
                            nc.vector.reciprocal(rz[h][:], z_sb[h][:])
                        if sc == 12:
                            zb = psB.tile([128, M], F32, name="zb",
                                          tag="zb")
                            nc.tensor.matmul(zb[:], ones_row[:], rz[h][:],
                                             start=True, stop=True,
                                             skip_group_check=True)
                            nc.vector.tensor_copy(zb_sb[h][:], zb[:])
                    nc.vector.tensor_mul(o2s[h][:], o_ps[:], zb_sb[h][:])
                    nc.sync.dma_start(o2_loc[h][:], o2s[h][:])
                    if h == 0:
                        # O-proj weights ride the SP queue while the
                        # AllGather is in flight
                        nc.sync.dma_start(wopa[:], wop[:])
                    nc.gpsimd.collective_compute(
                        "AllGather", mybir.AluOpType.bypass,
                        replica_groups=rg,
                        ins=[o2_loc[h].opt()], outs=[o2_all[h].opt()])
                    # gathered-o2 readback (SP queue is idle by now)
                    for k in range(8):
                        nc.sync.dma_start(
                            o2at[h][:, k * M:(k + 1) * M],
                            o2_all[h][k * 128:(k + 1) * 128, :])

            # ---------------- O projection (k-outer: h0 then h1) ----------
            with (
                tc.tile_pool(name="ps6", bufs=1, space="PSUM") as ps6,
                tc.tile_pool(name="s6o", bufs=3) as s6o,
            ):
                acc6 = [ps6.tile([128, M], F32, name=f"a6_{p}", tag=f"a6_{p}")
                        for p in range(7)]
                for n in range(16):
                    h, k = n // 8, n % 8
                    for p in range(7):
                        nc.tensor.matmul(
                            acc6[p][:],
                            wopa[:, n * OUT_C + p * 128:
                                 n * OUT_C + (p + 1) * 128],
                            o2at[h][:, k * M:(k + 1) * M],
                            start=(n == 0), stop=(n == 15))
                        if n == 15:
                            # evict each column block as soon as its last
                            # k-tile lands, overlapping the remaining mms
                            ob = s6o.tile([128, M], F16, name="outb",
                                          tag="outb")
                            nc.vector.tensor_copy(ob[:], acc6[p][:])
                            nc.sync.dma_start(outT[p], ob[:])
            per_cm.__exit__(None, None, None)

    nc.compile()
    return nc


def prep_inputs(x, W_cqkv, W_uq, W_qk, kv_cache, W_o1, W_oproj, indices):
    x = np.asarray(x, np.float32)
    W_cqkv = np.asarray(W_cqkv, np.float32)
    W_uq = np.asarray(W_uq, np.float32)
    W_qk = np.asarray(W_qk, np.float32)
    kv_cache = np.asarray(kv_cache, np.float32)
    W_o1 = np.asarray(W_o1, np.float32)
    W_oproj = np.asarray(W_oproj, np.float32)
    indices = np.asarray(indices)

    xT = np.ascontiguousarray(x.T)                     # [7168, 512]
    xtf = np.ascontiguousarray(
        xT.reshape(KB4, 4, 128, M).transpose(0, 2, 1, 3).reshape(
            KB4, 128, 2048)).astype(NP16)
    W_eff = W_cqkv[:, 512:512 + 1536] @ W_uq           # [7168, 3072]

    kvT = kv_cache.T                                   # [576, 4096]
    kvt5 = np.empty((NSC, 5, 128, 128), np.float16)
    for sc in range(NSC):
        blk = kvT[:, sc * 128:(sc + 1) * 128]
        for j in range(4):
            kvt5[sc, j] = blk[j * 128:(j + 1) * 128].astype(NP16)
        pe = blk[512:576].astype(NP16)
        kvt5[sc, 4, 0:64] = pe
        kvt5[sc, 4, 64:128] = pe
    kvt = np.ascontiguousarray(
        kvt5.transpose(0, 2, 1, 3).reshape(NSC, 128, 640))

    cm = np.zeros((M, S_KV), np.float32)
    np.add.at(cm, (np.arange(M)[:, None], indices), 1.0)
    cnt = np.ascontiguousarray(
        cm.T.reshape(NSC // 2, 2, 128, M).transpose(0, 2, 1, 3).reshape(
            NSC // 2, 128, 1024)).astype(NP16)

    V = kv_cache[:, :512]                              # [4096, 512]
    in_maps = []
    for i in range(N_CORES):
        g0, g1 = 2 * i, 2 * i + 1
        wef = np.concatenate([
            W_eff[:, g0 * 192:g0 * 192 + 128],
            W_eff[:, g1 * 192:g1 * 192 + 128],
            W_eff[:, g0 * 192 + 128:(g0 + 1) * 192],
            W_eff[:, g1 * 192 + 128:(g1 + 1) * 192],
        ], axis=1).astype(NP16).reshape(KB4, 4, 128, QL)
        weff = np.ascontiguousarray(
            wef.transpose(0, 2, 1, 3).reshape(KB4, 128, 4 * QL))
        v2 = np.empty((H_LOC, 128, S_KV), np.float16)
        for hl, g in enumerate((g0, g1)):
            v2[hl] = (V @ W_o1[g]).reshape(NSC, 128, 128).transpose(
                1, 0, 2).reshape(128, S_KV).astype(NP16)
        wop_rows = []
        for h in range(H_LOC):
            for rank in range(8):
                g = rank * H_LOC + h
                wop_rows.append(W_oproj[g * 128:(g + 1) * 128,
                                        i * OUT_C:(i + 1) * OUT_C])
        wopa = np.ascontiguousarray(
            np.stack(wop_rows).transpose(1, 0, 2).reshape(
                128, 16 * OUT_C)).astype(NP16)
        in_maps.append({
            "xt": xtf,
            "wef": weff,
            "wqk": W_qk[g0:g1 + 1].astype(NP16),
            "kvt": kvt,
            "cnt": cnt,
            "v2": v2,
            "wop": wopa,
        })
    return in_maps


_prog_cache = {}


def kernel(x, W_cqkv, W_uq, W_qk, kv_cache, W_o1, W_oproj, indices):
    if "nc" not in _prog_cache:
        _prog_cache["nc"] = build_program()
    nc = _prog_cache["nc"]
    in_maps = prep_inputs(x, W_cqkv, W_uq, W_qk, kv_cache, W_o1, W_oproj,
                          indices)
    trace = bool(int(os.environ.get("KERNEL_TRACE", "0")))
    res = run_bass_kernel_spmd(nc, in_maps, list(range(N_CORES)),
                               trace=trace)
    _prog_cache["last_result"] = res
    out = np.empty((M, HID), np.float32)
    for i in range(N_CORES):
        outT = res.results[i]["outT"].reshape(OUT_C, M)
        out[:, i * OUT_C:(i + 1) * OUT_C] = outT.T
    return out


# revision 31
# speedup vs baseline: 1.0319x; 1.0319x over previous
"""DeepSeek MLA prefill (absorbed) on 8 Trainium2 NeuronCores.

v3c: host-folded weights, head-sequential pipelined attention, batched
multi-queue DMA.

- W_uq folded through W_cqkv on host (W_eff = Wq @ W_uq): q for the local
  2 heads is one local GEMM against replicated x — no AllReduce; the first
  collective (o2 AllGather, head 0) fires mid-kernel, hiding the CC entry
  barrier entirely.
- W_o1 folded into V on host (V2[h] = V @ W_o1[h], [4096,128]): the value
  matmul contracts straight to o2 (4x less PE work, no o2 bmm).
- Attention head-sequential: head 0 scores->value->o2->AllGather, then
  head 1 (kv chunks stay SBUF-resident for the second pass); O-projection
  accumulates head-0 k-tiles while head 1's AllGather is in flight.
- Z is a running DVE accumulate over score chunks; one small f32 matmul
  folds it across partitions per head.
- DMA dispatch is serialized per issuing engine (~0.6us each), so
  transfers are batched into few big host-interleaved blocks and spread
  over the three dispatch paths: kv stream on SP, cnt stream on ACT,
  weight prefetches on GpSimd SWDGE.
- Top-k selection is folded in as a count matrix: softmax over gathered
  scores == count-weighted dense softmax against the full kv cache.
"""

import os
import sys

sys.path.insert(0, "/opt/trn_rl_repo")

import numpy as np

import concourse.bass as bass
import concourse.tile as tile
from concourse import bacc, mybir
from concourse.bass_utils import run_bass_kernel_spmd

F32 = mybir.dt.float32
F16 = mybir.dt.float16
NP16 = np.float16

N_CORES = 8
M = 512
HID = 7168
KB4 = HID // 512         # 14 blocks of 4 k-tiles
QL = 384                 # 2 local heads x (128 nope + 64 pe)
H_LOC = 2
S_KV = 4096
NSC = S_KV // 128        # 32 key chunks
OUT_C = HID // N_CORES   # 896
SM_SCALE = 1.0 / float(np.sqrt(np.float32(576)))
N_WARM = 20


def build_program():
    nc = bacc.Bacc("TRN2", target_bir_lowering=False, debug=False,
                   num_devices=N_CORES)

    xt = nc.dram_tensor("xt", [KB4, 128, 2048], F16, kind="ExternalInput")
    wef = nc.dram_tensor("wef", [KB4, 128, 4 * QL], F16,
                         kind="ExternalInput")
    wqk = nc.dram_tensor("wqk", [H_LOC, 128, 512], F16, kind="ExternalInput")
    # per chunk [128, 640]: 4 d-tiles + pe tile (rows 0:64 pe, 64:128 dup)
    kvt = nc.dram_tensor("kvt", [NSC, 128, 640], F16, kind="ExternalInput")
    cnt = nc.dram_tensor("cnt", [NSC // 2, 128, 1024], F16,
                         kind="ExternalInput")
    v2 = nc.dram_tensor("v2", [H_LOC, 128, S_KV], F16, kind="ExternalInput")
    wop = nc.dram_tensor("wop", [128, 16 * OUT_C], F16, kind="ExternalInput")
    outT = nc.dram_tensor("outT", [7, 128, M], F16, kind="ExternalOutput")

    rg = [list(range(N_CORES))]

    with tile.TileContext(nc) as tc, \
            nc.allow_low_precision(reason="fp16 matmul pipeline"):
        with tc.tile_pool(name="dram", bufs=1, space="DRAM") as dram:
            o2_loc = [dram.tile([128, M], F16, name=f"o2loc{h}")
                      for h in range(H_LOC)]
            o2_all = [dram.tile([128 * N_CORES, M], F16, name=f"o2all{h}",
                                addr_space="Shared") for h in range(H_LOC)]

            per_cm = tc.tile_pool(name="per", bufs=1)
            per = per_cm.__enter__()
            wqkt = []
            for h in range(H_LOC):
                wh = per.tile([128, 512], F16, name=f"wqk{h}")
                nc.sync.dma_start(wh[:], wqk[h])
                wqkt.append(wh)
            kvda = per.tile([128, NSC * 640], F16, name="kvda")
            pt = per.tile([128, NSC * M], F16, name="pt")
            qa = [[None] * 5 for _ in range(H_LOC)]
            zacc = [per.tile([128, M], F16, name=f"zacc{h}")
                    for h in range(H_LOC)]
            ones_col_f = per.tile([128, 1], F32, name="ones_col_f")
            nc.vector.memset(ones_col_f[:], 1.0)
            ones_col = per.tile([128, 1], F16, name="ones_col")
            nc.vector.tensor_copy(ones_col[:], ones_col_f[:])
            ones_row_f = per.tile([1, 128], F32, name="ones_row_f")
            nc.vector.memset(ones_row_f[:], 1.0)
            ones_row = per.tile([1, 128], F16, name="ones_row")
            nc.vector.tensor_copy(ones_row[:], ones_row_f[:])
            z_sb = [per.tile([1, M], F32, name=f"z{h}")
                    for h in range(H_LOC)]
            rz = [per.tile([1, M], F16, name=f"rz{h}") for h in range(H_LOC)]
            zb_sb = [per.tile([128, M], F16, name=f"zs{h}")
                     for h in range(H_LOC)]
            v2t = [per.tile([128, S_KV], F16, name=f"v2t{h}")
                   for h in range(H_LOC)]
            wopa = per.tile([128, 16 * OUT_C], F16, name="wopa")
            o2s = [per.tile([128, M], F16, name=f"o2s{h}")
                   for h in range(H_LOC)]
            o2at = [per.tile([128, 8 * M], F16, name=f"o2at{h}")
                    for h in range(H_LOC)]

            # ---------------- fused q GEMM (stage 1+2) --------------------
            qch = []
            with (
                tc.tile_pool(name="s12w", bufs=1) as s12w,
                tc.tile_pool(name="s12x", bufs=5) as s12x,
                tc.tile_pool(name="s12e", bufs=5) as s12e,
                tc.tile_pool(name="ps12", bufs=1, space="PSUM") as ps12,
            ):
                warm = s12w.tile([128, 64], F32, name="warm")
                nc.vector.memset(warm[:], 0.0)
                wps = ps12.tile([1, 64], F32, name="wps", tag="wps")
                for i in range(N_WARM):
                    nc.tensor.matmul(wps[:], warm[:, 0:1], warm[:],
                                     start=(i == 0), stop=(i == N_WARM - 1),
                                     skip_group_check=True)
                acc12 = [ps12.tile([128, M], F32, name=f"a12_{p}",
                                   tag=f"a12_{p}") for p in range(3)]
                for k4 in range(KB4):
                    xk = s12x.tile([128, 2048], F16, name="xk", tag="xk")
                    nc.sync.dma_start(xk[:], xt[k4])
                    ek = s12e.tile([128, 4 * QL], F16, name="ek", tag="ek")
                    nc.sync.dma_start(ek[:], wef[k4])
                    for q in range(4):
                        for p in range(3):
                            nc.tensor.matmul(
                                acc12[p][:],
                                ek[:, q * QL + p * 128:q * QL + (p + 1) * 128],
                                xk[:, q * 512:(q + 1) * 512],
                                start=(k4 == 0 and q == 0),
                                stop=(k4 == KB4 - 1 and q == 3))
                for p in range(3):
                    qc = per.tile([128, M], F16, name=f"qch{p}")
                    nc.vector.tensor_copy(qc[:], acc12[p][:])
                    qch.append(qc)
            for h in range(H_LOC):
                qa[h][4] = qch[2][h * 64:(h + 1) * 64, :]

            # ---------------- q absorb (stage 3) --------------------------
            with tc.tile_pool(name="ps3", bufs=2, space="PSUM") as ps3:
                for h in range(H_LOC):
                    for c in range(4):
                        acc = ps3.tile([128, M], F32, name="acc3", tag="acc3")
                        nc.tensor.matmul(
                            acc[:], wqkt[h][:, c * 128:(c + 1) * 128],
                            qch[h][:], start=True, stop=True)
                        qb = per.tile([128, M], F16, name=f"qa{h}_{c}")
                        nc.vector.tensor_copy(qb[:], acc[:])
                        qa[h][c] = qb

            # ---------------- attention + O path, head-sequential ---------
            with (
                tc.tile_pool(name="cnts", bufs=3) as cnts,
                tc.tile_pool(name="exps", bufs=5) as exps,
                tc.tile_pool(name="psS", bufs=4, space="PSUM") as psS,
                tc.tile_pool(name="psV", bufs=1, space="PSUM") as psV,
                tc.tile_pool(name="psZ", bufs=1, space="PSUM") as psZ,
                tc.tile_pool(name="psB", bufs=1, space="PSUM") as psB,
            ):
                for h in range(H_LOC):
                    cc2 = None
                    for sc in range(NSC):
                        if h == 0:
                            # stream kv chunk into the resident tile via SP;
                            # head 1 reuses it from SBUF
                            nc.sync.dma_start(
                                kvda[:, sc * 640:(sc + 1) * 640], kvt[sc])
                        if sc % 2 == 0:
                            # cnt pairs stream on the ACT hwdge queue
                            cc2 = cnts.tile([128, 1024], F16, name="cc",
                                            tag="cc")
                            nc.scalar.dma_start(cc2[:], cnt[sc // 2])
                        cc = cc2[:, (sc % 2) * 512:(sc % 2 + 1) * 512]
                        acc = psS.tile([128, M], F32, name="accS", tag="accS")
                        for j in range(5):
                            if j < 4:
                                lhsT = kvda[:, sc * 640 + j * 128:
                                            sc * 640 + (j + 1) * 128]
                                rhs = qa[h][j][:]
                            else:
                                lhsT = kvda[h * 64:(h + 1) * 64,
                                            sc * 640 + 512:sc * 640 + 640]
                                rhs = qa[h][4]
                            nc.tensor.matmul(
                                acc[:], lhsT, rhs,
                                start=(j == 0), stop=(j == 4))
                        ex = exps.tile([128, M], F16, name="ex", tag="ex")
                        nc.scalar.activation(
                            ex[:], acc[:], mybir.ActivationFunctionType.Exp,
                            scale=SM_SCALE)
                        psl = pt[:, sc * M:(sc + 1) * M]
                        nc.vector.tensor_mul(psl, ex[:], cc)
                        if sc == 0:
                            nc.vector.tensor_copy(zacc[h][:], psl)
                        else:
                            nc.vector.tensor_add(zacc[h][:], zacc[h][:], psl)
                        if h == 0 and sc == NSC - 1:
                            # v2 prefetch on ACT; transfers run during the
                            # HBM-quiet value phase, sliced so the value
                            # matmuls can chase the stream
                            for hh in range(H_LOC):
                                for s4 in range(4):
                                    nc.scalar.dma_start(
                                        v2t[hh][:,
                                                s4 * 1024:(s4 + 1) * 1024],
                                        v2[hh][:,
                                               s4 * 1024:(s4 + 1) * 1024])

                    # value phase; the Z fold rides along once the DVE
                    # accumulate chain has drained
                    o_ps = psV.tile([128, M], F32, name=f"op{h}",
                                    tag=f"vh{h}")
                    zp = psZ.tile([1, M], F32, name="zp", tag="zp")
                    for sc in range(NSC):
                        nc.tensor.matmul(
                            o_ps[:], v2t[h][:, sc * 128:(sc + 1) * 128],
                            pt[:, sc * M:(sc + 1) * M],
                            start=(sc == 0), stop=(sc == NSC - 1),
                            skip_group_check=True)
                        if sc == 1:
                            nc.tensor.matmul(zp[:], ones_col[:],
                                             zacc[h][:], start=True,
                                             stop=True,
                                             skip_group_check=True)
                            nc.vector.tensor_copy(z_sb[h][:], zp[:])
                            nc.vector.reciprocal(rz[h][:], z_sb[h][:])
                        if sc == 24:
                            zb = psB.tile([128, M], F32, name="zb",
                                          tag="zb")
                            nc.tensor.matmul(zb[:], ones_row[:], rz[h][:],
                                             start=True, stop=True,
                                             skip_group_check=True)
                            nc.vector.tensor_copy(zb_sb[h][:], zb[:])
                    nc.vector.tensor_mul(o2s[h][:], o_ps[:], zb_sb[h][:])
                    nc.sync.dma_start(o2_loc[h][:], o2s[h][:])
                    if h == 0:
                        # O-proj weights ride the SP queue while the
                        # AllGather is in flight
                        nc.sync.dma_start(wopa[:], wop[:])
                    nc.gpsimd.collective_compute(
                        "AllGather", mybir.AluOpType.bypass,
                        replica_groups=rg,
                        ins=[o2_loc[h].opt()], outs=[o2_all[h].opt()])
                    # gathered-o2 readback (SP queue is idle by now)
                    for k in range(8):
                        nc.sync.dma_start(
                            o2at[h][:, k * M:(k + 1) * M],
                            o2_all[h][k * 128:(k + 1) * 128, :])

            # ---------------- O projection (k-outer: h0 then h1) ----------
            with (
                tc.tile_pool(name="ps6", bufs=1, space="PSUM") as ps6,
                tc.tile_pool(name="s6o", bufs=3) as s6o,
            ):
                acc6 = [ps6.tile([128, M], F32, name=f"a6_{p}", tag=f"a6_{p}")
                        for p in range(7)]
                for n in range(16):
                    h, k = n // 8, n % 8
                    for p in range(7):
                        nc.tensor.matmul(
                            acc6[p][:],
                            wopa[:, n * OUT_C + p * 128:
                                 n * OUT_C + (p + 1) * 128],
                            o2at[h][:, k * M:(k + 1) * M],
                            start=(n == 0), stop=(n == 15))
                        if n == 15:
                            # evict each column block as soon as its last
                            # k-tile lands, overlapping the remaining mms
                            ob = s6o.tile([128, M], F16, name="outb",
                                          tag="outb")
                            nc.vector.tensor_copy(ob[:], acc6[p][:])
                            nc.sync.dma_start(outT[p], ob[:])
            per_cm.__exit__(None, None, None)

    nc.compile()
    return nc


def prep_inputs(x, W_cqkv, W_uq, W_qk, kv_cache, W_o1, W_oproj, indices):
    x = np.asarray(x, np.float32)
    W_cqkv = np.asarray(W_cqkv, np.float32)
    W_uq = np.asarray(W_uq, np.float32)
    W_qk = np.asarray(W_qk, np.float32)
    kv_cache = np.asarray(kv_cache, np.float32)
    W_o1 = np.asarray(W_o1, np.float32)
    W_oproj = np.asarray(W_oproj, np.float32)
    indices = np.asarray(indices)

    xT = np.ascontiguousarray(x.T)                     # [7168, 512]
    xtf = np.ascontiguousarray(
        xT.reshape(KB4, 4, 128, M).transpose(0, 2, 1, 3).reshape(
            KB4, 128, 2048)).astype(NP16)
    W_eff = W_cqkv[:, 512:512 + 1536] @ W_uq           # [7168, 3072]

    kvT = kv_cache.T                                   # [576, 4096]
    kvt5 = np.empty((NSC, 5, 128, 128), np.float16)
    for sc in range(NSC):
        blk = kvT[:, sc * 128:(sc + 1) * 128]
        for j in range(4):
            kvt5[sc, j] = blk[j * 128:(j + 1) * 128].astype(NP16)
        pe = blk[512:576].astype(NP16)
        kvt5[sc, 4, 0:64] = pe
        kvt5[sc, 4, 64:128] = pe
    kvt = np.ascontiguousarray(
        kvt5.transpose(0, 2, 1, 3).reshape(NSC, 128, 640))

    cm = np.zeros((M, S_KV), np.float32)
    np.add.at(cm, (np.arange(M)[:, None], indices), 1.0)
    cnt = np.ascontiguousarray(
        cm.T.reshape(NSC // 2, 2, 128, M).transpose(0, 2, 1, 3).reshape(
            NSC // 2, 128, 1024)).astype(NP16)

    V = kv_cache[:, :512]                              # [4096, 512]
    in_maps = []
    for i in range(N_CORES):
        g0, g1 = 2 * i, 2 * i + 1
        wef = np.concatenate([
            W_eff[:, g0 * 192:g0 * 192 + 128],
            W_eff[:, g1 * 192:g1 * 192 + 128],
            W_eff[:, g0 * 192 + 128:(g0 + 1) * 192],
            W_eff[:, g1 * 192 + 128:(g1 + 1) * 192],
        ], axis=1).astype(NP16).reshape(KB4, 4, 128, QL)
        weff = np.ascontiguousarray(
            wef.transpose(0, 2, 1, 3).reshape(KB4, 128, 4 * QL))
        v2 = np.empty((H_LOC, 128, S_KV), np.float16)
        for hl, g in enumerate((g0, g1)):
            v2[hl] = (V @ W_o1[g]).reshape(NSC, 128, 128).transpose(
                1, 0, 2).reshape(128, S_KV).astype(NP16)
        wop_rows = []
        for h in range(H_LOC):
            for rank in range(8):
                g = rank * H_LOC + h
                wop_rows.append(W_oproj[g * 128:(g + 1) * 128,
                                        i * OUT_C:(i + 1) * OUT_C])
        wopa = np.ascontiguousarray(
            np.stack(wop_rows).transpose(1, 0, 2).reshape(
                128, 16 * OUT_C)).astype(NP16)
        in_maps.append({
            "xt": xtf,
            "wef": weff,
            "wqk": W_qk[g0:g1 + 1].astype(NP16),
            "kvt": kvt,
            "cnt": cnt,
            "v2": v2,
            "wop": wopa,
        })
    return in_maps


_prog_cache = {}


def kernel(x, W_cqkv, W_uq, W_qk, kv_cache, W_o1, W_oproj, indices):
    if "nc" not in _prog_cache:
        _prog_cache["nc"] = build_program()
    nc = _prog_cache["nc"]
    in_maps = prep_inputs(x, W_cqkv, W_uq, W_qk, kv_cache, W_o1, W_oproj,
                          indices)
    trace = bool(int(os.environ.get("KERNEL_TRACE", "0")))
    res = run_bass_kernel_spmd(nc, in_maps, list(range(N_CORES)),
                               trace=trace)
    _prog_cache["last_result"] = res
    out = np.empty((M, HID), np.float32)
    for i in range(N_CORES):
        outT = res.results[i]["outT"].reshape(OUT_C, M)
        out[:, i * OUT_C:(i + 1) * OUT_C] = outT.T
    return out


# revision 32
# speedup vs baseline: 1.0889x; 1.0553x over previous
"""DeepSeek MLA prefill (absorbed) on 8 Trainium2 NeuronCores.

v3c: host-folded weights, head-sequential pipelined attention, batched
multi-queue DMA.

- W_uq folded through W_cqkv on host (W_eff = Wq @ W_uq): q for the local
  2 heads is one local GEMM against replicated x — no AllReduce; the first
  collective (o2 AllGather, head 0) fires mid-kernel, hiding the CC entry
  barrier entirely.
- W_o1 folded into V on host (V2[h] = V @ W_o1[h], [4096,128]): the value
  matmul contracts straight to o2 (4x less PE work, no o2 bmm).
- Attention head-sequential: head 0 scores->value->o2->AllGather, then
  head 1 (kv chunks stay SBUF-resident for the second pass); O-projection
  accumulates head-0 k-tiles while head 1's AllGather is in flight.
- Z is a running DVE accumulate over score chunks; one small f32 matmul
  folds it across partitions per head.
- DMA dispatch is serialized per issuing engine (~0.6us each), so
  transfers are batched into few big host-interleaved blocks and spread
  over the three dispatch paths: kv stream on SP, cnt stream on ACT,
  weight prefetches on GpSimd SWDGE.
- Top-k selection is folded in as a count matrix: softmax over gathered
  scores == count-weighted dense softmax against the full kv cache.
"""

import os
import sys

sys.path.insert(0, "/opt/trn_rl_repo")

import numpy as np

import concourse.bass as bass
import concourse.tile as tile
from concourse import bacc, mybir
from concourse.bass_utils import run_bass_kernel_spmd

F32 = mybir.dt.float32
F16 = mybir.dt.float16
NP16 = np.float16

N_CORES = 8
M = 512
HID = 7168
KB4 = HID // 512         # 14 blocks of 4 k-tiles
QL = 384                 # 2 local heads x (128 nope + 64 pe)
H_LOC = 2
S_KV = 4096
NSC = S_KV // 128        # 32 key chunks
OUT_C = HID // N_CORES   # 896
SM_SCALE = 1.0 / float(np.sqrt(np.float32(576)))
N_WARM = 20


def build_program():
    nc = bacc.Bacc("TRN2", target_bir_lowering=False, debug=False,
                   num_devices=N_CORES)

    xt = nc.dram_tensor("xt", [KB4, 128, 2048], F16, kind="ExternalInput")
    wef = nc.dram_tensor("wef", [KB4, 128, 4 * QL], F16,
                         kind="ExternalInput")
    wqk = nc.dram_tensor("wqk", [H_LOC, 128, 512], F16, kind="ExternalInput")
    # per chunk [128, 640]: 4 d-tiles + pe tile (rows 0:64 pe, 64:128 dup)
    kvt = nc.dram_tensor("kvt", [NSC, 128, 640], F16, kind="ExternalInput")
    cnt = nc.dram_tensor("cnt", [NSC // 2, 128, 1024], F16,
                         kind="ExternalInput")
    v2 = nc.dram_tensor("v2", [H_LOC, 128, S_KV], F16, kind="ExternalInput")
    wop = nc.dram_tensor("wop", [128, 16 * OUT_C], F16, kind="ExternalInput")
    outT = nc.dram_tensor("outT", [7, 128, M], F16, kind="ExternalOutput")

    rg = [list(range(N_CORES))]

    with tile.TileContext(nc) as tc, \
            nc.allow_low_precision(reason="fp16 matmul pipeline"):
        with tc.tile_pool(name="dram", bufs=1, space="DRAM") as dram:
            o2_loc = [dram.tile([128, M], F16, name=f"o2loc{h}")
                      for h in range(H_LOC)]
            o2_all = [dram.tile([128 * N_CORES, M], F16, name=f"o2all{h}",
                                addr_space="Shared") for h in range(H_LOC)]

            per_cm = tc.tile_pool(name="per", bufs=1)
            per = per_cm.__enter__()
            wqkt = []
            for h in range(H_LOC):
                wh = per.tile([128, 512], F16, name=f"wqk{h}")
                nc.sync.dma_start(wh[:], wqk[h])
                wqkt.append(wh)
            kvda = per.tile([128, NSC * 640], F16, name="kvda")
            pt = per.tile([128, NSC * M], F16, name="pt")
            qa = [[None] * 5 for _ in range(H_LOC)]
            zacc = [per.tile([128, M], F16, name=f"zacc{h}")
                    for h in range(H_LOC)]
            ones_col_f = per.tile([128, 1], F32, name="ones_col_f")
            nc.vector.memset(ones_col_f[:], 1.0)
            ones_col = per.tile([128, 1], F16, name="ones_col")
            nc.vector.tensor_copy(ones_col[:], ones_col_f[:])
            ones_row_f = per.tile([1, 128], F32, name="ones_row_f")
            nc.vector.memset(ones_row_f[:], 1.0)
            ones_row = per.tile([1, 128], F16, name="ones_row")
            nc.vector.tensor_copy(ones_row[:], ones_row_f[:])
            z_sb = [per.tile([1, M], F32, name=f"z{h}")
                    for h in range(H_LOC)]
            rz = [per.tile([1, M], F16, name=f"rz{h}") for h in range(H_LOC)]
            zb_sb = [per.tile([128, M], F16, name=f"zs{h}")
                     for h in range(H_LOC)]
            v2t = [per.tile([128, S_KV], F16, name=f"v2t{h}")
                   for h in range(H_LOC)]
            wopa = per.tile([128, 16 * OUT_C], F16, name="wopa")
            o2s = [per.tile([128, M], F16, name=f"o2s{h}")
                   for h in range(H_LOC)]
            o2at = [per.tile([128, 8 * M], F16, name=f"o2at{h}")
                    for h in range(H_LOC)]

            # ---------------- fused q GEMM (stage 1+2) --------------------
            qch = []
            with (
                tc.tile_pool(name="s12w", bufs=1) as s12w,
                tc.tile_pool(name="s12x", bufs=5) as s12x,
                tc.tile_pool(name="s12e", bufs=5) as s12e,
                tc.tile_pool(name="ps12", bufs=1, space="PSUM") as ps12,
            ):
                warm = s12w.tile([128, 64], F32, name="warm")
                nc.vector.memset(warm[:], 0.0)
                wps = ps12.tile([1, 64], F32, name="wps", tag="wps")
                for i in range(N_WARM):
                    nc.tensor.matmul(wps[:], warm[:, 0:1], warm[:],
                                     start=(i == 0), stop=(i == N_WARM - 1),
                                     skip_group_check=True)
                acc12 = [ps12.tile([128, M], F32, name=f"a12_{p}",
                                   tag=f"a12_{p}") for p in range(3)]
                for k4 in range(KB4):
                    xk = s12x.tile([128, 2048], F16, name="xk", tag="xk")
                    nc.sync.dma_start(xk[:], xt[k4])
                    ek = s12e.tile([128, 4 * QL], F16, name="ek", tag="ek")
                    nc.sync.dma_start(ek[:], wef[k4])
                    for q in range(4):
                        for p in range(3):
                            nc.tensor.matmul(
                                acc12[p][:],
                                ek[:, q * QL + p * 128:q * QL + (p + 1) * 128],
                                xk[:, q * 512:(q + 1) * 512],
                                start=(k4 == 0 and q == 0),
                                stop=(k4 == KB4 - 1 and q == 3))
                for p in range(3):
                    qc = per.tile([128, M], F16, name=f"qch{p}")
                    nc.vector.tensor_copy(qc[:], acc12[p][:])
                    qch.append(qc)
            for h in range(H_LOC):
                qa[h][4] = qch[2][h * 64:(h + 1) * 64, :]

            # ---------------- q absorb (stage 3) --------------------------
            with tc.tile_pool(name="ps3", bufs=2, space="PSUM") as ps3:
                for h in range(H_LOC):
                    for c in range(4):
                        acc = ps3.tile([128, M], F32, name="acc3", tag="acc3")
                        nc.tensor.matmul(
                            acc[:], wqkt[h][:, c * 128:(c + 1) * 128],
                            qch[h][:], start=True, stop=True)
                        qb = per.tile([128, M], F16, name=f"qa{h}_{c}")
                        nc.vector.tensor_copy(qb[:], acc[:])
                        qa[h][c] = qb

            # ---------------- attention + O path, head-sequential ---------
            with (
                tc.tile_pool(name="cnts", bufs=7) as cnts,
                tc.tile_pool(name="exps", bufs=5) as exps,
                tc.tile_pool(name="psS", bufs=4, space="PSUM") as psS,
                tc.tile_pool(name="psV", bufs=1, space="PSUM") as psV,
                tc.tile_pool(name="psZ", bufs=1, space="PSUM") as psZ,
                tc.tile_pool(name="psB", bufs=1, space="PSUM") as psB,
            ):
                for h in range(H_LOC):
                    ccp = []
                    for j in range(4):
                        t = cnts.tile([128, 1024], F16, name="cc", tag="cc")
                        nc.scalar.dma_start(t[:], cnt[j])
                        ccp.append(t)
                    for sc in range(NSC):
                        if h == 0:
                            # stream kv chunk into the resident tile via SP;
                            # head 1 reuses it from SBUF
                            nc.sync.dma_start(
                                kvda[:, sc * 640:(sc + 1) * 640], kvt[sc])
                        if sc % 2 == 0 and sc + 8 < NSC:
                            # stay 4 pairs ahead on the ACT hwdge queue
                            t = cnts.tile([128, 1024], F16, name="cc",
                                          tag="cc")
                            nc.scalar.dma_start(t[:], cnt[(sc + 8) // 2])
                            ccp.append(t)
                        cc = ccp[sc // 2][:, (sc % 2) * 512:
                                          (sc % 2 + 1) * 512]
                        acc = psS.tile([128, M], F32, name="accS", tag="accS")
                        for j in range(5):
                            if j < 4:
                                lhsT = kvda[:, sc * 640 + j * 128:
                                            sc * 640 + (j + 1) * 128]
                                rhs = qa[h][j][:]
                            else:
                                lhsT = kvda[h * 64:(h + 1) * 64,
                                            sc * 640 + 512:sc * 640 + 640]
                                rhs = qa[h][4]
                            nc.tensor.matmul(
                                acc[:], lhsT, rhs,
                                start=(j == 0), stop=(j == 4))
                        ex = exps.tile([128, M], F16, name="ex", tag="ex")
                        nc.scalar.activation(
                            ex[:], acc[:], mybir.ActivationFunctionType.Exp,
                            scale=SM_SCALE)
                        psl = pt[:, sc * M:(sc + 1) * M]
                        nc.vector.tensor_mul(psl, ex[:], cc)
                        if sc == 0:
                            nc.vector.tensor_copy(zacc[h][:], psl)
                        else:
                            nc.vector.tensor_add(zacc[h][:], zacc[h][:], psl)
                        if h == 0 and sc == NSC - 1:
                            # v2 prefetch on ACT; transfers run during the
                            # HBM-quiet value phase, sliced so the value
                            # matmuls can chase the stream
                            for hh in range(H_LOC):
                                for s4 in range(4):
                                    nc.scalar.dma_start(
                                        v2t[hh][:,
                                                s4 * 1024:(s4 + 1) * 1024],
                                        v2[hh][:,
                                               s4 * 1024:(s4 + 1) * 1024])

                    # value phase; the Z fold rides along once the DVE
                    # accumulate chain has drained
                    o_ps = psV.tile([128, M], F32, name=f"op{h}",
                                    tag=f"vh{h}")
                    zp = psZ.tile([1, M], F32, name="zp", tag="zp")
                    for sc in range(NSC):
                        nc.tensor.matmul(
                            o_ps[:], v2t[h][:, sc * 128:(sc + 1) * 128],
                            pt[:, sc * M:(sc + 1) * M],
                            start=(sc == 0), stop=(sc == NSC - 1),
                            skip_group_check=True)
                        if sc == 1:
                            nc.tensor.matmul(zp[:], ones_col[:],
                                             zacc[h][:], start=True,
                                             stop=True,
                                             skip_group_check=True)
                            nc.vector.tensor_copy(z_sb[h][:], zp[:])
                            nc.vector.reciprocal(rz[h][:], z_sb[h][:])
                        if sc == 24:
                            zb = psB.tile([128, M], F32, name="zb",
                                          tag="zb")
                            nc.tensor.matmul(zb[:], ones_row[:], rz[h][:],
                                             start=True, stop=True,
                                             skip_group_check=True)
                            nc.vector.tensor_copy(zb_sb[h][:], zb[:])
                    nc.vector.tensor_mul(o2s[h][:], o_ps[:], zb_sb[h][:])
                    nc.sync.dma_start(o2_loc[h][:], o2s[h][:])
                    if h == 0:
                        # O-proj weights ride the SP queue while the
                        # AllGather is in flight
                        nc.sync.dma_start(wopa[:], wop[:])
                    nc.gpsimd.collective_compute(
                        "AllGather", mybir.AluOpType.bypass,
                        replica_groups=rg,
                        ins=[o2_loc[h].opt()], outs=[o2_all[h].opt()])
                    # gathered-o2 readback (SP queue is idle by now)
                    for k in range(8):
                        nc.sync.dma_start(
                            o2at[h][:, k * M:(k + 1) * M],
                            o2_all[h][k * 128:(k + 1) * 128, :])

            # ---------------- O projection (k-outer: h0 then h1) ----------
            with (
                tc.tile_pool(name="ps6", bufs=1, space="PSUM") as ps6,
                tc.tile_pool(name="s6o", bufs=3) as s6o,
            ):
                acc6 = [ps6.tile([128, M], F32, name=f"a6_{p}", tag=f"a6_{p}")
                        for p in range(7)]
                for n in range(16):
                    h, k = n // 8, n % 8
                    for p in range(7):
                        nc.tensor.matmul(
                            acc6[p][:],
                            wopa[:, n * OUT_C + p * 128:
                                 n * OUT_C + (p + 1) * 128],
                            o2at[h][:, k * M:(k + 1) * M],
                            start=(n == 0), stop=(n == 15))
                        if n == 15:
                            # evict each column block as soon as its last
                            # k-tile lands, overlapping the remaining mms
                            ob = s6o.tile([128, M], F16, name="outb",
                                          tag="outb")
                            nc.vector.tensor_copy(ob[:], acc6[p][:])
                            nc.sync.dma_start(outT[p], ob[:])
            per_cm.__exit__(None, None, None)

    nc.compile()
    return nc


def prep_inputs(x, W_cqkv, W_uq, W_qk, kv_cache, W_o1, W_oproj, indices):
    x = np.asarray(x, np.float32)
    W_cqkv = np.asarray(W_cqkv, np.float32)
    W_uq = np.asarray(W_uq, np.float32)
    W_qk = np.asarray(W_qk, np.float32)
    kv_cache = np.asarray(kv_cache, np.float32)
    W_o1 = np.asarray(W_o1, np.float32)
    W_oproj = np.asarray(W_oproj, np.float32)
    indices = np.asarray(indices)

    xT = np.ascontiguousarray(x.T)                     # [7168, 512]
    xtf = np.ascontiguousarray(
        xT.reshape(KB4, 4, 128, M).transpose(0, 2, 1, 3).reshape(
            KB4, 128, 2048)).astype(NP16)
    W_eff = W_cqkv[:, 512:512 + 1536] @ W_uq           # [7168, 3072]

    kvT = kv_cache.T                                   # [576, 4096]
    kvt5 = np.empty((NSC, 5, 128, 128), np.float16)
    for sc in range(NSC):
        blk = kvT[:, sc * 128:(sc + 1) * 128]
        for j in range(4):
            kvt5[sc, j] = blk[j * 128:(j + 1) * 128].astype(NP16)
        pe = blk[512:576].astype(NP16)
        kvt5[sc, 4, 0:64] = pe
        kvt5[sc, 4, 64:128] = pe
    kvt = np.ascontiguousarray(
        kvt5.transpose(0, 2, 1, 3).reshape(NSC, 128, 640))

    cm = np.zeros((M, S_KV), np.float32)
    np.add.at(cm, (np.arange(M)[:, None], indices), 1.0)
    cnt = np.ascontiguousarray(
        cm.T.reshape(NSC // 2, 2, 128, M).transpose(0, 2, 1, 3).reshape(
            NSC // 2, 128, 1024)).astype(NP16)

    V = kv_cache[:, :512]                              # [4096, 512]
    in_maps = []
    for i in range(N_CORES):
        g0, g1 = 2 * i, 2 * i + 1
        wef = np.concatenate([
            W_eff[:, g0 * 192:g0 * 192 + 128],
            W_eff[:, g1 * 192:g1 * 192 + 128],
            W_eff[:, g0 * 192 + 128:(g0 + 1) * 192],
            W_eff[:, g1 * 192 + 128:(g1 + 1) * 192],
        ], axis=1).astype(NP16).reshape(KB4, 4, 128, QL)
        weff = np.ascontiguousarray(
            wef.transpose(0, 2, 1, 3).reshape(KB4, 128, 4 * QL))
        v2 = np.empty((H_LOC, 128, S_KV), np.float16)
        for hl, g in enumerate((g0, g1)):
            v2[hl] = (V @ W_o1[g]).reshape(NSC, 128, 128).transpose(
                1, 0, 2).reshape(128, S_KV).astype(NP16)
        wop_rows = []
        for h in range(H_LOC):
            for rank in range(8):
                g = rank * H_LOC + h
                wop_rows.append(W_oproj[g * 128:(g + 1) * 128,
                                        i * OUT_C:(i + 1) * OUT_C])
        wopa = np.ascontiguousarray(
            np.stack(wop_rows).transpose(1, 0, 2).reshape(
                128, 16 * OUT_C)).astype(NP16)
        in_maps.append({
            "xt": xtf,
            "wef": weff,
            "wqk": W_qk[g0:g1 + 1].astype(NP16),
            "kvt": kvt,
            "cnt": cnt,
            "v2": v2,
            "wop": wopa,
        })
    return in_maps


_prog_cache = {}


def kernel(x, W_cqkv, W_uq, W_qk, kv_cache, W_o1, W_oproj, indices):
    if "nc" not in _prog_cache:
        _prog_cache["nc"] = build_program()
    nc = _prog_cache["nc"]
    in_maps = prep_inputs(x, W_cqkv, W_uq, W_qk, kv_cache, W_o1, W_oproj,
                          indices)
    trace = bool(int(os.environ.get("KERNEL_TRACE", "0")))
    res = run_bass_kernel_spmd(nc, in_maps, list(range(N_CORES)),
                               trace=trace)
    _prog_cache["last_result"] = res
    out = np.empty((M, HID), np.float32)
    for i in range(N_CORES):
        outT = res.results[i]["outT"].reshape(OUT_C, M)
        out[:, i * OUT_C:(i + 1) * OUT_C] = outT.T
    return out
